# revision 1
# baseline (speedup 1.0000x reference)
"""Single-head causal attention on 8 TRN2 NeuronCores, batch-parallel.

Per core (1 batch element): x [2048,1024] f32, Wq/Wk/Wv [1024,64] f32.
  qkT = [Wq|Wk]^T @ x^T   (fused projection, f32r matmuls)
  ST[s,t] = k_s . q_t     (scores computed transposed, K=64)
  P = exp(ST/8), causal-masked via 0/1 mask tiles (no max-subtraction:
      inputs are bounded, |S| < ~7, exp cannot overflow)
  OT[h,t] = sum_s v'[s,h] P[s,t] with v' = [v | 1] so row 64 = softmax
      denominator; final O = (OT[:64]/OT[64]).T
"""
import numpy as np

import concourse.bass as bass
import concourse.mybir as mybir
import concourse.tile as tile
from concourse import bacc, bass_utils
from concourse.masks import make_identity

P = 128
T = 2048
C = 1024
H = 64
NT = T // P      # 16 t-blocks
NCC = C // P     # 8 c-chunks
F32 = mybir.dt.float32
F32R = mybir.dt.float32r
SCALE = 1.0 / np.sqrt(H)

_CACHE = {}


def build_program(trace_friendly=False):
    nc = bacc.Bacc("TRN2", target_bir_lowering=False, debug=False, num_devices=8)
    # x declared f32r: same 4-byte storage; PE transposes run at 1.5 vs 2
    # cycles/row and feed f32r projections directly.
    x_d = nc.dram_tensor("x", [T, C], F32R, kind="ExternalInput").ap()
    wq_d = nc.dram_tensor("Wq", [C, H], F32, kind="ExternalInput").ap()
    wk_d = nc.dram_tensor("Wk", [C, H], F32, kind="ExternalInput").ap()
    wv_d = nc.dram_tensor("Wv", [C, H], F32, kind="ExternalInput").ap()
    o_d = nc.dram_tensor("out", [T, H], F32, kind="ExternalOutput").ap()

    with tile.TileContext(nc) as tc:
        _body(nc, tc, x_d, wq_d, wk_d, wv_d, o_d)
    nc.compile()
    try:
        build_program.last_perfetto = tc._perfetto_entries
    except Exception:
        build_program.last_perfetto = None
    return nc


def _body(nc, tc, x_d, wq_d, wk_d, wv_d, o_d):
    from contextlib import ExitStack
    ctx = ExitStack()
    with ctx:
        consts = ctx.enter_context(tc.tile_pool(name="consts", bufs=1))
        big = ctx.enter_context(tc.tile_pool(name="big", bufs=1))
        xin = ctx.enter_context(tc.tile_pool(name="xin", bufs=3))
        ptile = ctx.enter_context(tc.tile_pool(name="ptile", bufs=4))

        # ---- constants ----
        ident = consts.tile([P, P], F32)
        make_identity(nc, ident[:])
        ident_r = consts.tile([P, P], F32R, tag="ident_r")
        nc.vector.tensor_copy(ident_r[:], ident[:])
        # mask01[d]: [128,512] f32r, keep (1.0) where t_local + off - s >= 0
        masks = []
        for d in range(4):
            mf = consts.tile([P, 512], F32, tag=f"maskf{d}")
            nc.gpsimd.memset(mf[:], 1.0)
            nc.gpsimd.affine_select(
                out=mf[:], in_=mf[:], compare_op=mybir.AluOpType.is_ge,
                fill=0.0, base=-d * P, channel_multiplier=-1,
                pattern=[[1, 512]],
            )
            m = consts.tile([P, 512], F32R, tag=f"mask{d}")
            nc.vector.tensor_copy(m[:], mf[:])
            masks.append(m)

        # ---- weights: [C,H] -> [128, 8, 64], fuse q|k, convert to f32r ----
        w_stage = {}
        for name, ap in (("q", wq_d), ("k", wk_d), ("v", wv_d)):
            ws = consts.tile([P, NCC, H], F32, tag=f"ws_{name}")
            nc.sync.dma_start(ws[:], ap.rearrange("(cc p) h -> p cc h", p=P))
            w_stage[name] = ws
        w_qk = consts.tile([P, NCC, P], F32R, tag="w_qk")
        nc.vector.tensor_copy(w_qk[:, :, 0:H], w_stage["q"][:])
        nc.vector.tensor_copy(w_qk[:, :, H:P], w_stage["k"][:])
        w_v = consts.tile([P, NCC, H], F32R, tag="w_v")
        nc.vector.tensor_copy(w_v[:], w_stage["v"][:])

        # ---- persistent big tensors ----
        xT = big.tile([P, NCC, T], F32R, tag="xT")        # x^T, c on partitions
        qkT = big.tile([P, T], F32R, tag="qkT")           # rows 0:64 qT, 64:128 kT
        kT = big.tile([H, T], F32R, tag="kT")             # kT at partitions 0:63
        vTs = big.tile([H, T], F32, tag="vT")             # v^T [h, s]
        v1 = big.tile([P, NT, H + 1], F32R, tag="v1")     # v natural + ones col
        o_sb = big.tile([P, NT, H], F32, tag="o")         # final output

        # ---- phase 1+2: load x, transpose, project ----
        with tc.tile_pool(name="psA", bufs=4, space="PSUM") as psA, \
             tc.tile_pool(name="psQK", bufs=2, space="PSUM") as psQK, \
             tc.tile_pool(name="psV", bufs=2, space="PSUM") as psV:
            x_r = x_d.rearrange("(n p) c -> p n c", p=P)  # [128, 16, 1024]
            for tg in range(4):                           # t-groups of 512
                xb = xin.tile([P, 4, C], F32R, tag="xb")
                nc.sync.dma_start(xb[:], x_r[:, tg * 4:(tg + 1) * 4, :])
                for cc in range(NCC):
                    ps = psA.tile([P, 512], F32R, tag="xp")
                    for i in range(4):
                        nc.tensor.transpose(
                            ps[:, i * P:(i + 1) * P],
                            xb[:, i, cc * P:(cc + 1) * P], ident_r[:])
                    # PSUM->SBUF copy converts to f32r; alternate ACT/DVE
                    dst = xT[:, cc, tg * 512:(tg + 1) * 512]
                    if cc % 2 == 0:
                        nc.scalar.copy(dst, ps[:])
                    else:
                        nc.vector.tensor_copy(dst, ps[:])
                # projections for this 512-wide t-slice (keeps PE warm too)
                pqk = psQK.tile([P, 512], F32, tag="qk")
                pv = psV.tile([H, 512], F32, tag="v")
                for cc in range(NCC):
                    rhs = xT[:, cc, tg * 512:(tg + 1) * 512]
                    nc.tensor.matmul(pqk[:], w_qk[:, cc, :], rhs,
                                     start=(cc == 0), stop=(cc == NCC - 1))
                for cc in range(NCC):
                    rhs = xT[:, cc, tg * 512:(tg + 1) * 512]
                    nc.tensor.matmul(pv[:], w_v[:, cc, :], rhs,
                                     start=(cc == 0), stop=(cc == NCC - 1))
                nc.vector.tensor_copy(qkT[:, tg * 512:(tg + 1) * 512], pqk[:])
                nc.vector.tensor_copy(vTs[:, tg * 512:(tg + 1) * 512], pv[:])

            # kT to partitions 0:63 (cross-partition: SBUF->SBUF DMA)
            nc.sync.dma_start(kT[:], qkT[H:P, :])

            # v natural: PE-transpose vT in 128-col chunks
            for ss in range(NT):
                pvt = psA.tile([P, H], F32, tag="xp")
                nc.tensor.transpose(pvt[:], vTs[:, ss * P:(ss + 1) * P],
                                    ident[0:H, 0:H])
                nc.vector.tensor_copy(v1[:, ss, 0:H], pvt[:])
            ones_f = consts.tile([P, NT], F32, tag="ones_f")
        nc.gpsimd.memset(ones_f[:], 1.0)
        nc.vector.tensor_copy(v1[:, :, H], ones_f[:])

        # ---- phase 3: attention ----
        psB = ctx.enter_context(tc.tile_pool(name="psB", bufs=2, space="PSUM"))
        psOT = ctx.enter_context(tc.tile_pool(name="psOT", bufs=4, space="PSUM"))
        ot = [psOT.tile([H + 1, 512], F32, tag="ot", name=f"ot{i}")
              for i in range(4)]

        for j in range(NT):                  # key block (s = j*128 ...)
            for u in range(j // 8, 2):       # 1024-wide t tiles
                halves = [h for h in range(2)
                          if (u * 1024 + (h + 1) * 512) > j * P]
                st = psB.tile([P, 1024], F32, tag="st")
                for h in halves:
                    nc.tensor.matmul(
                        st[:, h * 512:(h + 1) * 512],
                        kT[:, j * P:(j + 1) * P],
                        qkT[0:H, u * 1024 + h * 512: u * 1024 + (h + 1) * 512],
                        start=True, stop=True)
                pt = ptile.tile([P, 1024], F32R, tag="pt")
                if len(halves) == 2:
                    nc.scalar.activation(pt[:], st[:],
                                         mybir.ActivationFunctionType.Exp,
                                         scale=SCALE)
                else:
                    h = halves[0]
                    nc.scalar.activation(pt[:, h * 512:(h + 1) * 512],
                                         st[:, h * 512:(h + 1) * 512],
                                         mybir.ActivationFunctionType.Exp,
                                         scale=SCALE)
                if u == j // 8:              # diagonal tile: causal mask
                    hd = (j % 8) // 4
                    sl = slice(hd * 512, (hd + 1) * 512)
                    nc.vector.tensor_mul(out=pt[:, sl], in0=pt[:, sl],
                                         in1=masks[j % 4][:])
                for h in halves:
                    tcn = u * 2 + h
                    nc.tensor.matmul(
                        ot[tcn][:], v1[:, j, :],
                        pt[:, h * 512:(h + 1) * 512],
                        start=(j == 0), stop=(j == 4 * tcn + 3))

        # ---- phase 4: normalize + transpose back ----
        otsb_pool = ctx.enter_context(tc.tile_pool(name="otsb", bufs=2))
        rec_pool = ctx.enter_context(tc.tile_pool(name="rec", bufs=4))
        for tcn in range(4):
            osb = otsb_pool.tile([H + 1, 512], F32, tag="otsb")
            nc.vector.tensor_copy(osb[:], ot[tcn][:])
            for q in range(4):
                po = psB.tile([P, H + 1], F32, tag="st")
                nc.tensor.transpose(po[:], osb[:, q * P:(q + 1) * P],
                                    ident[0:H + 1, 0:H + 1])
                rec = rec_pool.tile([P, 1], F32, tag="rec")
                nc.vector.reciprocal(rec[:], po[:, H:H + 1])
                nc.vector.tensor_scalar_mul(
                    o_sb[:, tcn * 4 + q, :], po[:, 0:H], rec[:])
        nc.sync.dma_start(o_d.rearrange("(n p) h -> p n h", p=P), o_sb[:])


def kernel(x, Wq, Wk, Wv):
    key = "prog"
    if key not in _CACHE:
        _CACHE[key] = build_program()
    nc = _CACHE[key]
    B = x.shape[0]
    in_maps = [{"x": np.ascontiguousarray(x[b], dtype=np.float32),
                "Wq": np.asarray(Wq, dtype=np.float32),
                "Wk": np.asarray(Wk, dtype=np.float32),
                "Wv": np.asarray(Wv, dtype=np.float32)} for b in range(B)]
    res = bass_utils.run_bass_kernel_spmd(nc, in_maps, list(range(B)))
    return np.stack([res.results[b]["out"] for b in range(B)], axis=0)


def run_traced(x, Wq, Wk, Wv):
    """Same as kernel() but with NTFF tracing; returns (out, BassKernelResults)."""
    nc = build_program()
    B = x.shape[0]
    in_maps = [{"x": np.ascontiguousarray(x[b], dtype=np.float32),
                "Wq": np.asarray(Wq, dtype=np.float32),
                "Wk": np.asarray(Wk, dtype=np.float32),
                "Wv": np.asarray(Wv, dtype=np.float32)} for b in range(B)]
    res = bass_utils.run_bass_kernel_spmd(nc, in_maps, list(range(B)),
                                          trace=True)
    out = np.stack([res.results[b]["out"] for b in range(B)], axis=0)
    return out, res



# revision 2
# speedup vs baseline: 23.9957x; 23.9957x over previous
"""Single-head causal attention (B=8, T=2048, C=1024, H=64) for TRN2.

Strategy: the per-execution runtime overhead in this environment dwarfs
the ~100us of actual compute, so all 8 batch elements are computed by ONE
NeuronCore inside a single NEFF execution (batch loop unrolled in the
program).  Throughput across the 8 cores comes from running independent
executions on different cores concurrently (see test.py).

Kernel math per batch element (identical to the tuned 8-core baseline):
  qkT = [Wq|Wk]^T @ x^T   (fused projection, f32r matmuls)
  ST[s,t] = k_s . q_t     (scores computed transposed, K=64)
  P = exp(ST/8), causal-masked via 0/1 mask tiles (no max-subtraction:
      inputs are bounded, |S| < ~7, exp cannot overflow)
  OT[h,t] = sum_s v'[s,h] P[s,t] with v' = [v | 1] so row 64 = softmax
      denominator; final O = (OT[:64]/OT[64]).T

Differences vs the baseline:
- Weights, causal masks, identity and ones are shipped in ONE fused
  input tensor `cw` [128, NCW] built host-side (fewer bound tensors,
  no gpsimd const-building, and the weight DMA is one contiguous
  transfer instead of 3x1024 strided 256B descriptors).
- Weight layout is c-blocked (chunk r, partition j holds c = j*8+r,
  i.e. a plain reshape of the [1024, 64] weight), matched by stride-8
  column slices in the x transposes: the contraction over c is a sum,
  so any consistent permutation of c is exact.
"""
import numpy as np
from contextlib import ExitStack

import concourse.bass as bass
import concourse.mybir as mybir
import concourse.tile as tile
from concourse import bacc, bass_utils

P = 128
T = 2048
C = 1024
H = 64
B = 8
NT = T // P      # 16
NCC = C // P     # 8
F32 = mybir.dt.float32
F32R = mybir.dt.float32r
SCALE = 1.0 / np.sqrt(H)

# cw column offsets
O_WQK = 0                     # [NCC, 128]  (cols 0:64 Wq, 64:128 Wk)
O_WV = O_WQK + NCC * P        # [NCC, 64]
O_MASK = O_WV + NCC * H       # 4 x [512]
O_ID = O_MASK + 4 * 512       # [128]
O_ONES = O_ID + P             # [16]
NCW = O_ONES + NT

_CACHE = {}


def make_cw(Wq, Wk, Wv):
    """[128, NCW] fused weights+constants tensor."""
    cw = np.zeros((P, NCW), dtype=np.float32)
    wq = np.asarray(Wq, np.float32).reshape(P, NCC, H)   # c = p*8+r
    wk = np.asarray(Wk, np.float32).reshape(P, NCC, H)
    wv = np.asarray(Wv, np.float32).reshape(P, NCC, H)
    wqk = cw[:, O_WQK:O_WV].reshape(P, NCC, P)
    wqk[:, :, 0:H] = wq
    wqk[:, :, H:P] = wk
    cw[:, O_WV:O_MASK] = wv.reshape(P, NCC * H)
    # mask d: [128 s-local, 512 t-local], keep where t - d*128 >= s
    s = np.arange(P)[:, None]
    t = np.arange(512)[None, :]
    for d in range(4):
        cw[:, O_MASK + d * 512:O_MASK + (d + 1) * 512] = (
            (t - d * P - s) >= 0).astype(np.float32)
    cw[:, O_ID:O_ID + P] = np.eye(P, dtype=np.float32)
    cw[:, O_ONES:O_ONES + NT] = 1.0
    return cw


def build_program(bpc=B):
    nc = bacc.Bacc("TRN2", target_bir_lowering=False, debug=False,
                   num_devices=max(B // bpc, 1))
    x_d = nc.dram_tensor("x", [bpc * T, C], F32R, kind="ExternalInput").ap()
    cw_d = nc.dram_tensor("cw", [P, NCW], F32R, kind="ExternalInput").ap()
    o_d = nc.dram_tensor("out", [bpc * T, H], F32, kind="ExternalOutput").ap()

    with tile.TileContext(nc) as tc:
        ctx = ExitStack()
        with ctx:
            consts = ctx.enter_context(tc.tile_pool(name="consts", bufs=1))
            big = ctx.enter_context(tc.tile_pool(name="big", bufs=1))
            xin = ctx.enter_context(tc.tile_pool(name="xin", bufs=3))
            ptile = ctx.enter_context(tc.tile_pool(name="ptile", bufs=4))

            cw = consts.tile([P, NCW], F32R, tag="cw")
            nc.sync.dma_start(cw[:], cw_d)
            w_qk = cw[:, O_WQK:O_WV].rearrange("p (r q) -> p r q", r=NCC)
            w_v = cw[:, O_WV:O_MASK].rearrange("p (r h) -> p r h", r=NCC)
            masks = [cw[:, O_MASK + d * 512:O_MASK + (d + 1) * 512]
                     for d in range(4)]
            ident_r = cw[:, O_ID:O_ID + P]
            ones_f = cw[:, O_ONES:O_ONES + NT]
            # f32-dtyped identity for the f32 transposes (same bits)
            ident = consts.tile([P, P], F32, tag="ident")
            nc.vector.tensor_copy(ident[:], ident_r)

            xT = big.tile([P, NCC, T], F32R, tag="xT")
            qkT = big.tile([P, T], F32R, tag="qkT")
            kT = big.tile([H, T], F32R, tag="kT")
            vTs = big.tile([H, T], F32, tag="vT")
            v1 = big.tile([P, NT, H + 1], F32R, tag="v1")
            o_sb = big.tile([P, NT, H], F32, tag="o")

            for b in range(bpc):
                _one_batch(nc, tc, x_d, o_d, b, ident, ident_r, masks,
                           w_qk, w_v, xT, qkT, kT, vTs, v1, o_sb, ones_f,
                           xin, ptile)
    nc.compile()
    try:
        build_program.last_perfetto = tc._perfetto_entries
    except Exception:
        build_program.last_perfetto = None
    return nc


def _one_batch(nc, tc, x_d, o_d, b, ident, ident_r, masks, w_qk, w_v,
               xT, qkT, kT, vTs, v1, o_sb, ones_f, xin, ptile):
    # ---- phase 1+2: load x, transpose (chunk r holds c = 8j+r), project ----
    x_r = x_d[b * T:(b + 1) * T, :].rearrange("(n p) c -> p n c", p=P)
    ph1 = ExitStack()
    psA = ph1.enter_context(tc.tile_pool(name=f"psA{b}", bufs=4, space="PSUM"))
    psQK = ph1.enter_context(tc.tile_pool(name=f"psQK{b}", bufs=2, space="PSUM"))
    for tg in range(4):
        xb = xin.tile([P, 4, C], F32R, tag="xb")
        nc.sync.dma_start(xb[:], x_r[:, tg * 4:(tg + 1) * 4, :])
        for cc in range(NCC):
            ps = psA.tile([P, 512], F32R, tag="xp")
            for i in range(4):
                nc.tensor.transpose(
                    ps[:, i * P:(i + 1) * P],
                    xb[:, i, cc::NCC], ident_r)
            dst = xT[:, cc, tg * 512:(tg + 1) * 512]
            if cc % 2 == 0:
                nc.scalar.copy(dst, ps[:])
            else:
                nc.vector.tensor_copy(dst, ps[:])
        pqk = psQK.tile([P, 512], F32, tag="qk")
        pv = psQK.tile([H, 512], F32, tag="v")
        for cc in range(NCC):
            rhs = xT[:, cc, tg * 512:(tg + 1) * 512]
            nc.tensor.matmul(pqk[:], w_qk[:, cc, :], rhs,
                             start=(cc == 0), stop=(cc == NCC - 1))
        for cc in range(NCC):
            rhs = xT[:, cc, tg * 512:(tg + 1) * 512]
            nc.tensor.matmul(pv[:], w_v[:, cc, :], rhs,
                             start=(cc == 0), stop=(cc == NCC - 1))
        nc.vector.tensor_copy(qkT[:, tg * 512:(tg + 1) * 512], pqk[:])
        nc.vector.tensor_copy(vTs[:, tg * 512:(tg + 1) * 512], pv[:])

    # kT to partitions 0:63 (cross-partition: SBUF->SBUF DMA)
    nc.sync.dma_start(kT[:], qkT[H:P, :])

    # v natural + ones column
    for ss in range(NT):
        pvt = psA.tile([P, H], F32, tag="xp")
        nc.tensor.transpose(pvt[:], vTs[:, ss * P:(ss + 1) * P],
                            ident[0:H, 0:H])
        nc.vector.tensor_copy(v1[:, ss, 0:H], pvt[:])
    nc.vector.tensor_copy(v1[:, :, H], ones_f)
    ph1.close()

    # ---- phase 3: attention ----
    ph3 = ExitStack()
    psB = ph3.enter_context(tc.tile_pool(name=f"psB{b}", bufs=2, space="PSUM"))
    psOT = ph3.enter_context(tc.tile_pool(name=f"psOT{b}", bufs=4, space="PSUM"))
    ot = [psOT.tile([H + 1, 512], F32, tag="ot", name=f"ot{i}_b{b}")
          for i in range(4)]
    for j in range(NT):                  # key block (s = j*128 ...)
        for u in range(j // 8, 2):       # 1024-wide t tiles
            halves = [h for h in range(2)
                      if (u * 1024 + (h + 1) * 512) > j * P]
            st = psB.tile([P, 1024], F32, tag="st")
            for h in halves:
                nc.tensor.matmul(
                    st[:, h * 512:(h + 1) * 512],
                    kT[:, j * P:(j + 1) * P],
                    qkT[0:H, u * 1024 + h * 512: u * 1024 + (h + 1) * 512],
                    start=True, stop=True)
            pt = ptile.tile([P, 1024], F32R, tag="pt")
            if len(halves) == 2:
                nc.scalar.activation(pt[:], st[:],
                                     mybir.ActivationFunctionType.Exp,
                                     scale=SCALE)
            else:
                h = halves[0]
                nc.scalar.activation(pt[:, h * 512:(h + 1) * 512],
                                     st[:, h * 512:(h + 1) * 512],
                                     mybir.ActivationFunctionType.Exp,
                                     scale=SCALE)
            if u == j // 8:              # diagonal tile: causal mask
                hd = (j % 8) // 4
                sl = slice(hd * 512, (hd + 1) * 512)
                nc.vector.tensor_mul(out=pt[:, sl], in0=pt[:, sl],
                                     in1=masks[j % 4])
            for h in halves:
                tcn = u * 2 + h
                nc.tensor.matmul(
                    ot[tcn][:], v1[:, j, :],
                    pt[:, h * 512:(h + 1) * 512],
                    start=(j == 0), stop=(j == 4 * tcn + 3))

    # ---- phase 4: normalize + transpose back ----
    for tcn in range(4):
        osb = xin.tile([H + 1, 512], F32, tag="otsb")
        nc.vector.tensor_copy(osb[:], ot[tcn][:])
        for q in range(4):
            po = psB.tile([P, H + 1], F32, tag="st")
            nc.tensor.transpose(po[:], osb[:, q * P:(q + 1) * P],
                                ident[0:H + 1, 0:H + 1])
            rec = ptile.tile([P, 1], F32, tag="rec")
            nc.vector.reciprocal(rec[:], po[:, H:H + 1])
            nc.vector.tensor_scalar_mul(
                o_sb[:, tcn * 4 + q, :], po[:, 0:H], rec[:])
    nc.sync.dma_start(
        o_d[b * T:(b + 1) * T, :].rearrange("(n p) h -> p n h", p=P), o_sb[:])
    ph3.close()


def kernel(x, Wq, Wk, Wv, bpc=B):
    key = f"prog{bpc}"
    if key not in _CACHE:
        _CACHE[key] = build_program(bpc)
    nc = _CACHE[key]
    nb = x.shape[0]
    ncores = nb // bpc
    cw = make_cw(Wq, Wk, Wv)
    in_maps = []
    for c in range(ncores):
        xs = np.ascontiguousarray(
            x[c * bpc:(c + 1) * bpc].reshape(bpc * T, C), dtype=np.float32)
        in_maps.append({"x": xs, "cw": cw})
    res = bass_utils.run_bass_kernel_spmd(nc, in_maps, list(range(ncores)))
    return np.concatenate(
        [res.results[c]["out"].reshape(bpc, T, H) for c in range(ncores)],
        axis=0)
